# revision 1
# baseline (speedup 1.0000x reference)
"""2-layer GCN (improved=True) + linear head + softmax on 8 Trainium2 cores.

Strategy (dest-node partitioning, per sharding hint):
- Nodes are assigned to 8 cores x 49 tiles x 128 slots via balanced bin-packing
  (each tile's in-edge count <= 18*128 so every tile processes exactly 18
  edge-chunks of 128 edges; self-loops are folded in as regular edges with
  norm = 2*dis^2).
- Per layer: each core computes XW for its node slots (node-major [n,128]),
  AllGather replicates the full table to every core's HBM, then per dest-tile
  the core gathers the 2304 source rows with dma_gather (two gathers from two
  overlapping table halves so indices fit int16) and scatter-adds them with
  one-hot matmuls: agg^T[d, n] += G_chunk^T[d, e] @ S_chunk[e, n], where
  S[e, n] = norm(e) if edge e's dest is tile-slot n. Bias+ReLU on the
  feat-major aggregate via ScalarE, which directly feeds the next layer's
  XW matmul as lhsT.
- Head: logits = H2 @ Wlin + blin, softmax over 8 classes, all on-chip.

kernel() is self-contained: host-side numpy does all graph preprocessing
(deg/norm, node assignment, chunking, S matrices, wrapped int16 gather
indices); the device program is identical on all 8 cores, only data differs.
"""
import sys

sys.path.insert(0, "/opt/trn_rl_repo")

import numpy as np
import ml_dtypes

import concourse.bass as bass
import concourse.bacc as bacc
import concourse.mybir as mybir
import concourse.tile as tile
from concourse.tile_rust import add_dep_helper
from concourse.bass_utils import run_bass_kernel_spmd
from concourse.library_config import mlp

# problem constants
N = 50000
E = 800000
FIN = 512
D = 128
NCLS = 8
NCORES = 8

# sharding constants
P = 128
TILES = 49
NLOC = TILES * P            # 6272 slots per core
VTOT = NCORES * NLOC        # 50176 table rows
CH = 9                      # chunks per half
CPT = 2 * CH                # 18 chunks per tile
NIH = CH * P                # 1152 gathered rows per half
WCOL = NIH // 16            # 72 int16 idx columns per half block
HI_BASE = 17408             # hi gather reads table[HI_BASE:], idx+32767 covers VTOT-1
TILES_A = 26                # tiles in AG phase A
ROWS_A = TILES_A * P        # 3328 rows/core in phase A
LO_LIM = NCORES * ROWS_A    # 26624: lo gathers read only the AG-A region
TILES_B = TILES - TILES_A
ROWS_B = TILES_B * P
PAIRS = (TILES + 1) // 2
DEPTH = 12                  # lo-gather software-pipeline depth (pairs)

TRACE = False
LAST_EXEC_NS = None
QN0 = 0
QN1 = 1
QROT = True

_PROGRAM = None


def _build_program():
    nc = bacc.Bacc(None, target_bir_lowering=False, num_swdge_queues=4)
    f32 = mybir.dt.float32
    bf16 = mybir.dt.bfloat16

    xt_d = nc.dram_tensor("xt", [FIN, NLOC], bf16, kind="ExternalInput")
    w1_d = nc.dram_tensor("w1", [FIN, D], bf16, kind="ExternalInput")
    w2_d = nc.dram_tensor("w2", [D, D], bf16, kind="ExternalInput")
    wl_d = nc.dram_tensor("wl", [D, NCLS], bf16, kind="ExternalInput")
    b1_d = nc.dram_tensor("b1", [P, 1], f32, kind="ExternalInput")
    b2_d = nc.dram_tensor("b2", [P, 1], f32, kind="ExternalInput")
    bl_d = nc.dram_tensor("bl", [P, NCLS], f32, kind="ExternalInput")
    gidx_d = nc.dram_tensor("gidx", [P, TILES * 2 * WCOL], mybir.dt.int16,
                            kind="ExternalInput")
    sval_d = nc.dram_tensor("sval", [TILES, P, CPT * P], bf16, kind="ExternalInput")
    out_d = nc.dram_tensor("probs", [NLOC, NCLS], f32, kind="ExternalOutput")

    with tile.TileContext(nc) as tc:
        lib = nc.gpsimd.load_library(mlp)
        first_gather = [True]

        with (
            tc.tile_pool(name="const", bufs=1) as cp,
            tc.tile_pool(name="xtp", bufs=1) as xtp,
            tc.tile_pool(name="t1sb", bufs=3) as t1p,
            tc.tile_pool(name="gpool", bufs=4) as gp,
            tc.tile_pool(name="spool", bufs=6) as sp,
            tc.tile_pool(name="hpool", bufs=3) as hp,
            tc.tile_pool(name="headp", bufs=3) as hdp,
            tc.tile_pool(name="xwps", bufs=2, space="PSUM") as xwps,
            tc.tile_pool(name="aggps", bufs=2, space="PSUM") as aggps,
            tc.tile_pool(name="lgps", bufs=2, space="PSUM") as lgps,
            tc.tile_pool(name="dram1", bufs=1, space="DRAM") as dr1,
            tc.tile_pool(name="dram2", bufs=1, space="DRAM") as dr2,
            tc.tile_pool(name="dram3", bufs=1, space="DRAM") as dr3,
            tc.tile_pool(name="dram4", bufs=1, space="DRAM") as dr4,
        ):
            # ---- constants to SBUF ----
            w1_sb = cp.tile([P, 4 * D], bf16)
            for k in range(4):
                nc.sync.dma_start(w1_sb[:, k * D:(k + 1) * D],
                                  w1_d[k * P:(k + 1) * P, :])
            w2_sb = cp.tile([P, D], bf16)
            nc.sync.dma_start(w2_sb[:], w2_d[:])
            wl_sb = cp.tile([P, NCLS], bf16)
            nc.sync.dma_start(wl_sb[:], wl_d[:])
            b1_sb = cp.tile([P, 1], f32)
            nc.sync.dma_start(b1_sb[:], b1_d[:])
            b2_sb = cp.tile([P, 1], f32)
            nc.sync.dma_start(b2_sb[:], b2_d[:])
            bl_sb = cp.tile([P, NCLS], f32)
            nc.sync.dma_start(bl_sb[:], bl_d[:])
            gidx_sb = cp.tile([P, TILES * 2 * WCOL], mybir.dt.int16)
            nc.sync.dma_start(gidx_sb[:], gidx_d[:])

            t_loc = [dr1.tile([NLOC, D], bf16, name="t_loc0"),
                     dr2.tile([NLOC, D], bf16, name="t_loc1")]
            t_full = [dr3.tile([VTOT, D], bf16, name="t_full0"),
                      dr4.tile([VTOT, D], bf16, name="t_full1")]

            # ---- phase 0: XW1 ----
            xt_sb = xtp.tile([P, 4 * NLOC], bf16)
            for k in range(4):
                nc.sync.dma_start(xt_sb[:, k * NLOC:(k + 1) * NLOC],
                                  xt_d[k * P:(k + 1) * P, :])
            for t in range(TILES):
                ps = xwps.tile([P, D], f32, tag="xw")
                for k in range(4):
                    nc.tensor.matmul(
                        out=ps[:],
                        lhsT=xt_sb[:, k * NLOC + t * P: k * NLOC + (t + 1) * P],
                        rhs=w1_sb[:, k * D:(k + 1) * D],
                        start=(k == 0), stop=(k == 3),
                    )
                tsb = t1p.tile([P, D], bf16, tag="t1")
                nc.scalar.activation(out=tsb[:], in_=ps[:],
                                     func=mybir.ActivationFunctionType.Copy)
                nc.sync.dma_start(t_loc[0][t * P:(t + 1) * P, :], tsb[:])

            def allgather(li):
                nc.gpsimd.collective_compute(
                    "AllGather",
                    mybir.AluOpType.bypass,
                    replica_groups=[list(range(NCORES))],
                    ins=[t_loc[li][0:ROWS_A, :].opt()],
                    outs=[t_full[li][0:LO_LIM, :].opt()],
                )
                nc.gpsimd.collective_compute(
                    "AllGather",
                    mybir.AluOpType.bypass,
                    replica_groups=[list(range(NCORES))],
                    ins=[t_loc[li][ROWS_A:NLOC, :].opt()],
                    outs=[t_full[li][LO_LIM:VTOT, :].opt()],
                )

            def gather_half(li, pp, nt, half):
                tag = "glo" if half == 0 else "ghi"
                nbuf = DEPTH + 1 if half == 0 else 3
                g = gp.tile([P, 2 * CH * D], bf16, tag=tag, bufs=nbuf,
                            name=f"g{tag}{li}_{pp}")
                off = pp * 4 * WCOL
                ni = nt * NIH
                src = (t_full[li][0:LO_LIM, :] if half == 0
                       else t_full[li][HI_BASE:VTOT, :])
                c0 = off + half * nt * WCOL
                gi = nc.gpsimd.dma_gather(
                    g[:, :nt * CH * D].rearrange("p (c d) -> p c d", d=D),
                    src,
                    gidx_sb[:, c0:c0 + nt * WCOL],
                    ni, ni, D, single_packet=False,
                    queue_num=(2 * pp + half) % 4 if QROT else (QN0 if half == 0 else QN1),
                )
                if first_gather[0]:
                    add_dep_helper(gi.ins, lib.ins, reason="lib before gather")
                    first_gather[0] = False
                return g

            def agg_tile(t, g_lo, g_hi, coff, s_sb):
                agg = aggps.tile([P, P], f32, tag="agg")
                for c in range(CPT):
                    g = g_lo if c < CH else g_hi
                    cc = coff + (c % CH)
                    nc.tensor.matmul(
                        out=agg[:],
                        lhsT=g[:, cc * D:(cc + 1) * D],
                        rhs=s_sb[:, c * P:(c + 1) * P],
                        start=(c == 0), stop=(c == CPT - 1),
                    )
                return agg

            # ---- phase 1+2: layer-1 aggregation + XW2 ----
            allgather(0)
            glo_buf = {}
            for pp in range(PAIRS + DEPTH):
                if pp < PAIRS:
                    nt = 2 if 2 * pp + 1 < TILES else 1
                    glo_buf[pp] = (gather_half(0, pp, nt, 0), nt)
                qq = pp - DEPTH
                if qq < 0:
                    continue
                g_lo, nt = glo_buf.pop(qq)
                g_hi = gather_half(0, qq, nt, 1)
                for ti in range(nt):
                    t = 2 * qq + ti
                    s_sb = sp.tile([P, CPT * P], bf16, tag="s")
                    nc.sync.dma_start(s_sb[:], sval_d[t, :, :])
                    agg = agg_tile(t, g_lo, g_hi, ti * CH, s_sb)
                    h1t = hp.tile([P, P], bf16, tag="h")
                    nc.scalar.activation(out=h1t[:], in_=agg[:],
                                         func=mybir.ActivationFunctionType.Relu,
                                         bias=b1_sb[:])
                    ps2 = xwps.tile([P, D], f32, tag="xw2")
                    nc.tensor.matmul(out=ps2[:], lhsT=h1t[:], rhs=w2_sb[:],
                                     start=True, stop=True)
                    t2sb = t1p.tile([P, D], bf16, tag="t2")
                    nc.vector.tensor_copy(out=t2sb[:], in_=ps2[:])
                    nc.sync.dma_start(t_loc[1][t * P:(t + 1) * P, :], t2sb[:])


            # ---- phase 3+4: layer-2 aggregation + head ----
            allgather(1)
            glo_buf = {}
            for pp in range(PAIRS + DEPTH):
                if pp < PAIRS:
                    nt = 2 if 2 * pp + 1 < TILES else 1
                    glo_buf[pp] = (gather_half(1, pp, nt, 0), nt)
                qq = pp - DEPTH
                if qq < 0:
                    continue
                g_lo, nt = glo_buf.pop(qq)
                g_hi = gather_half(1, qq, nt, 1)
                for ti in range(nt):
                    t = 2 * qq + ti
                    s_sb = sp.tile([P, CPT * P], bf16, tag="s")
                    nc.sync.dma_start(s_sb[:], sval_d[t, :, :])
                    agg = agg_tile(t, g_lo, g_hi, ti * CH, s_sb)
                    h2t = hp.tile([P, P], bf16, tag="h")
                    nc.scalar.activation(out=h2t[:], in_=agg[:],
                                         func=mybir.ActivationFunctionType.Relu,
                                         bias=b2_sb[:])
                    lg = lgps.tile([P, NCLS], f32, tag="lg")
                    nc.tensor.matmul(out=lg[:], lhsT=h2t[:], rhs=wl_sb[:],
                                     start=True, stop=True)
                    l_sb = hdp.tile([P, NCLS], f32, tag="l")
                    nc.vector.tensor_add(out=l_sb[:], in0=lg[:], in1=bl_sb[:])
                    nmx = hdp.tile([P, 1], f32, tag="nmx")
                    nc.vector.reduce_max(out=nmx[:], in_=l_sb[:],
                                         axis=mybir.AxisListType.X, negate=True)
                    e_sb = hdp.tile([P, NCLS], f32, tag="e")
                    nc.scalar.activation(out=e_sb[:], in_=l_sb[:],
                                         func=mybir.ActivationFunctionType.Exp,
                                         bias=nmx[:])
                    sm = hdp.tile([P, 1], f32, tag="sm")
                    nc.vector.reduce_sum(out=sm[:], in_=e_sb[:],
                                         axis=mybir.AxisListType.X)
                    rs = hdp.tile([P, 1], f32, tag="rs")
                    nc.vector.reciprocal(out=rs[:], in_=sm[:])
                    pr = hdp.tile([P, NCLS], f32, tag="pr")
                    nc.scalar.activation(out=pr[:], in_=e_sb[:],
                                         func=mybir.ActivationFunctionType.Copy,
                                         scale=rs[:])
                    nc.sync.dma_start(out_d[t * P:(t + 1) * P, :], pr[:])


    nc.compile()
    return nc


def _preprocess(x, edge_index, W1, b1, W2, b2, Wlin, blin):
    """Host-side graph preprocessing -> per-core input dicts + slot maps."""
    x = np.asarray(x, np.float32)
    ei = np.asarray(edge_index)
    row = ei[0].astype(np.int64)
    col = ei[1].astype(np.int64)

    deg = np.bincount(col, minlength=N).astype(np.float32) + 2.0
    dis = 1.0 / np.sqrt(deg)
    norm_e = dis[row] * dis[col]

    # append self-loops as regular edges
    allrow = np.concatenate([row, np.arange(N, dtype=np.int64)])
    allcol = np.concatenate([col, np.arange(N, dtype=np.int64)])
    allnorm = np.concatenate([norm_e, 2.0 * dis * dis]).astype(np.float32)

    indeg = np.bincount(allcol, minlength=N)  # per-node in-edges incl. self

    # balanced node->bin assignment (bins = core*TILES + tile), snake by degree
    NB = NCORES * TILES
    order = np.argsort(-indeg, kind="stable")
    bin_of_node = np.empty(N, np.int64)
    pos_in_bin = np.empty(N, np.int64)
    full_rounds = N // NB
    rem = N - full_rounds * NB
    fwd = np.arange(NB)
    bwd = fwd[::-1]
    seq = []
    for r in range(full_rounds):
        seq.append(fwd if r % 2 == 0 else bwd)
    if rem:
        seq.append((fwd if full_rounds % 2 == 0 else bwd)[:rem])
    seq = np.concatenate(seq)
    bin_of_node[order] = seq
    srt = np.argsort(bin_of_node, kind="stable")
    cnt = np.bincount(bin_of_node, minlength=NB)
    assert cnt.max() <= P
    starts = np.zeros(NB + 1, np.int64)
    np.cumsum(cnt, out=starts[1:])
    pos_in_bin[srt] = np.arange(N) - starts[bin_of_node[srt]]

    edge_cap = CPT * P
    bin_edge_cnt = np.bincount(bin_of_node[allcol], minlength=NB)
    assert bin_edge_cnt.max() <= edge_cap, (
        f"bin edge overflow: {bin_edge_cnt.max()} > {edge_cap}")

    core_of_node = bin_of_node // TILES
    tile_of_node = bin_of_node % TILES
    # table row: AG-A region holds tiles 0..TILES_A-1 of every core, then AG-B
    gslot = np.where(
        tile_of_node < TILES_A,
        core_of_node * ROWS_A + tile_of_node * P + pos_in_bin,
        LO_LIM + core_of_node * ROWS_B + (tile_of_node - TILES_A) * P + pos_in_bin,
    )

    # per-edge: destination bin + dest position; source table slot
    e_bin = bin_of_node[allcol]
    e_dpos = pos_in_bin[allcol]
    e_src = gslot[allrow]

    # group edges by bin
    e_order = np.argsort(e_bin, kind="stable")
    eb = e_bin[e_order]
    ed = e_dpos[e_order]
    es = e_src[e_order]
    en = allnorm[e_order]
    bstarts = np.searchsorted(eb, np.arange(NB + 1))

    in_maps = []
    for c in range(NCORES):
        gidx = np.zeros((P, TILES * 2 * WCOL), np.int16)
        sval = np.zeros((TILES, P, CPT * P), ml_dtypes.bfloat16)
        flats = np.zeros((TILES, 2, NIH), np.int64)  # [tile, half, pos]
        for t in range(TILES):
            b = c * TILES + t
            lo_f, hi_f = bstarts[b], bstarts[b + 1]
            srcs = es[lo_f:hi_f]
            dpos = ed[lo_f:hi_f]
            nrm = en[lo_f:hi_f]
            nlo_fix = int((srcs < HI_BASE).sum())
            ne = len(srcs)
            lo_n = max(nlo_fix, ne - NIH)
            assert lo_n <= NIH and ne - lo_n <= NIH
            # order: fixed-lo, then flex, then fixed-hi; flex split at lo_n
            cls = np.where(srcs < HI_BASE, 0, np.where(srcs >= LO_LIM, 2, 1))
            o2 = np.argsort(cls, kind="stable")
            srcs, dpos, nrm = srcs[o2], dpos[o2], nrm[o2]
            for half in (0, 1):
                if half == 0:
                    hs, hd, hn = srcs[:lo_n], dpos[:lo_n], nrm[:lo_n]
                    rel = hs
                else:
                    hs, hd, hn = srcs[lo_n:], dpos[lo_n:], nrm[lo_n:]
                    rel = hs - HI_BASE
                k = len(hs)
                o3 = np.argsort(rel, kind="stable")
                rel, hd2, hn2 = rel[o3], hd[o3], hn[o3]
                flats[t, half, :k] = rel
                # chunk-major positions: i -> chunk i//128, partition i%128
                ii = np.arange(k)
                cidx = ii // P + (0 if half == 0 else CH)
                pidx = ii % P
                sval[t, pidx, cidx * P + hd2] = hn2
        # gather-index blocks per tile PAIR: [pair: lo(A|B), hi(A|B)]
        for pp in range((TILES + 1) // 2):
            nt = 2 if 2 * pp + 1 < TILES else 1
            off = pp * 4 * WCOL
            for half in (0, 1):
                flat = np.concatenate(
                    [flats[2 * pp + ti, half] for ti in range(nt)])
                w = flat.reshape(len(flat) // 16, 16).T.astype(np.int16)
                c0 = off + half * nt * WCOL
                gidx[:, c0:c0 + nt * WCOL] = np.tile(w, (8, 1))
        # x slice, transposed, padded
        xt = np.zeros((FIN, NLOC), ml_dtypes.bfloat16)
        mine = np.where(core_of_node == c)[0]
        lslot = tile_of_node[mine] * P + pos_in_bin[mine]
        xt[:, lslot] = x[mine].T.astype(ml_dtypes.bfloat16)
        in_maps.append({
            "xt": xt,
            "w1": np.asarray(W1).astype(ml_dtypes.bfloat16),
            "w2": np.asarray(W2).astype(ml_dtypes.bfloat16),
            "wl": np.asarray(Wlin).astype(ml_dtypes.bfloat16),
            "b1": np.asarray(b1, np.float32).reshape(P, 1),
            "b2": np.asarray(b2, np.float32).reshape(P, 1),
            "bl": np.tile(np.asarray(blin, np.float32).reshape(1, NCLS), (P, 1)),
            "gidx": gidx,
            "sval": sval,
        })
    return in_maps, core_of_node, tile_of_node, pos_in_bin


def kernel(x, edge_index, W1, b1, W2, b2, Wlin, blin):
    global _PROGRAM, LAST_EXEC_NS
    in_maps, core_of, tile_of, pos_of = _preprocess(
        x, edge_index, W1, b1, W2, b2, Wlin, blin)
    if _PROGRAM is None:
        _PROGRAM = _build_program()
    res = run_bass_kernel_spmd(
        _PROGRAM, in_maps, core_ids=list(range(NCORES)), trace=TRACE)
    LAST_EXEC_NS = res.exec_time_ns
    out = np.empty((N, NCLS), np.float32)
    per_core = [res.results[c]["probs"] for c in range(NCORES)]
    lslot = tile_of * P + pos_of
    for c in range(NCORES):
        mine = np.where(core_of == c)[0]
        out[mine] = per_core[c][lslot[mine]]
    return out

